# revision 1
# baseline (speedup 1.0000x reference)
"""Trainium2 Bass kernel for nn_DisenGCNLayer (disentangled GCN layer), v2.

Strategy (8 NeuronCores, zero inter-core communication):
  - Destination nodes sharded across cores (49 tiles of 128 nodes each); the
    4 routing iterations of a tile touch only the tile's own h_dst state plus
    the fixed normalized source table, so tiles are fully independent.
  - Per tile, edges live in 128-wide chunks (edge-major layout). Segment
    softmax needs no max (unit-vector dots, |s|<=1); the denominator is
    factored out of the aggregation, so denom+agg is ONE PSUM-accumulated
    matmul per chunk against a gathered 0/1 onehot (fp8) matrix.
  - ACT uses only {exp, sqrt, square, relu, copy}: 1/x and 1/sqrt(x) run on
    the DVE reciprocal_approx_fast custom op, and group phases are
    interleaved so the exp<->sqrt table swap happens twice per round.
  - Gathers batched per group of 2 tiles, 3 groups in flight with
    prefetched inits: h_src via two int16 dma_gather calls (table split at
    remapped row 32768), onehot rows from a tiny fp8 identity table
    (matmul lhsT consumes fp8 directly), per-iteration h_dst expansion from
    a group bounce buffer with paired 512B descriptors. Odd-degree pad
    slots add exp(0)=1 to the denominator; a per-node pad count is
    subtracted before 1/denom. Node-update math stays fp32 (bf16 there
    amplifies through the iterated normalization).
  - h16/out are partition-major so bulk DMA uses >=512B descriptors;
    phase 0 consumes fp32 x/W (bf16 inputs perturb source directions
    enough to break the error gate at sensitive nodes).
"""

import heapq

import numpy as np
import ml_dtypes

import concourse.bass as bass
import concourse.bacc as bacc
import concourse.mybir as mybir
import concourse.tile as tile
from concourse.bass_utils import run_bass_kernel_spmd

bf16 = ml_dtypes.bfloat16
f8e4 = ml_dtypes.float8_e4m3

# problem spec (hardcoded)
N_NODES = 50000
N_EDGES = 800000
F = 128
K = 8
D = 16
ITERS = 4

NCORES = 8
P = 128
TILES = 392                 # total node tiles
TPC = TILES // NCORES       # 49 tiles per core
NPC = TPC * P               # 6272 nodes per core
NPAD = TILES * P            # 50176
NB = NPAD // P              # 392 column blocks in partition-major h16

NGROUPS = 25                # tile groups per core (24x2 + 1x1 slots)
FLIGHT = 3                  # groups processed concurrently
PGRP = 4                    # phase-0 node-chunk group size (512 nodes)

TRACE = False
DEBUG_STAGE = 99            # 1=phase0 only
LAST_RESULTS = {}


def _remap(n):
    """node id -> partition-major flat row of h16 viewed [P*NB, F]."""
    return (n % P) * NB + n // P


# --------------------------------------------------------------------------
# host-side preprocessing
# --------------------------------------------------------------------------

def _preprocess(edge_index):
    row = np.asarray(edge_index[0], dtype=np.int64).astype(np.int32)
    col = np.asarray(edge_index[1], dtype=np.int64).astype(np.int32)

    rows_of_col = (col.astype(np.int64) % P) * NB + col.astype(np.int64) // P
    isB_e = rows_of_col >= 32768
    degA = np.bincount(row[~isB_e], minlength=NPAD).astype(np.int64)
    degB = np.bincount(row[isB_e], minlength=NPAD).astype(np.int64)
    pdeg = degA + (degA & 1) + degB + (degB & 1)   # per-node per-side padding

    # --- bin-pack nodes into 392 tiles of exactly 128, balancing padded load
    order = np.argsort(-pdeg, kind="stable")
    heap = [(0, t) for t in range(TILES)]
    heapq.heapify(heap)
    tile_nodes = [[] for _ in range(TILES)]
    tile_load = np.zeros(TILES, np.int64)
    for n in order:
        while True:
            load, t = heapq.heappop(heap)
            if len(tile_nodes[t]) < P:
                break
        tile_nodes[t].append(n)
        tile_load[t] = load + pdeg[n]
        if len(tile_nodes[t]) < P:
            heapq.heappush(heap, (tile_load[t], t))

    # --- snake-assign tiles to cores by load; per-core sort by load desc so
    # slot j is similar across cores (C per slot is maxed across cores).
    t_order = np.argsort(-tile_load, kind="stable")
    core_tiles = [[] for _ in range(NCORES)]
    for i, t in enumerate(t_order):
        c = i % (2 * NCORES)
        c = c if c < NCORES else 2 * NCORES - 1 - c
        core_tiles[c].append(t)
    for c in range(NCORES):
        core_tiles[c].sort(key=lambda t: -tile_load[t])

    # per-slot, per-side chunk count: even, maxed across cores
    # (side = remapped-row < 32768, the int16 gather-index split)
    def _remap_side_load(t):
        a0, a1 = 0, 0  # placeholder; computed after bounds exist
        return 0
    CsA = np.zeros(TPC, np.int64)
    CsB = np.zeros(TPC, np.int64)

    # --- snake slots into NGROUPS buckets so sum(C) per group is uniform
    group_slots = [[] for _ in range(NGROUPS)]
    for r in range((TPC + NGROUPS - 1) // NGROUPS):
        for b in range(NGROUPS):
            j = r * NGROUPS + (b if r % 2 == 0 else NGROUPS - 1 - b)
            if j < TPC:
                group_slots[b].append(j)

    # --- per-node tile assignment + local slot
    node_tile = np.empty(NPAD, np.int32)
    node_local = np.empty(NPAD, np.int32)
    for t in range(TILES):
        ids = np.sort(np.array(tile_nodes[t], np.int64))
        tile_nodes[t] = ids
        node_tile[ids] = t
        node_local[ids] = np.arange(P, dtype=np.int32)

    # --- group edges by tile
    ekey = node_tile[row].astype(np.int64)
    eorder = np.argsort(ekey, kind="stable")
    col_s = col[eorder]
    seg_s = node_local[row[eorder]]
    bounds = np.searchsorted(ekey[eorder], np.arange(TILES + 1))

    # per-tile padded load per side
    rows_s = _remap(col_s.astype(np.int64))
    isB_s = rows_s >= 32768
    loadA = np.zeros(TILES, np.int64)
    loadB = np.zeros(TILES, np.int64)
    for t in range(TILES):
        a0, a1 = bounds[t], bounds[t + 1]
        segs = seg_s[a0:a1]
        bmask = isB_s[a0:a1]
        ca = np.bincount(segs[~bmask], minlength=P)
        cb = np.bincount(segs[bmask], minlength=P)
        loadA[t] = (ca + (ca & 1)).sum()
        loadB[t] = (cb + (cb & 1)).sum()
    for j in range(TPC):
        ma = max(loadA[core_tiles[c][j]] for c in range(NCORES))
        mb = max(loadB[core_tiles[c][j]] for c in range(NCORES))
        ca = -(-int(ma) // P)
        cb = -(-int(mb) // P)
        CsA[j] = ca + (ca & 1)
        CsB[j] = cb + (cb & 1)

    SPLIT = 32768
    DEADA = int(_remap(50000))   # zero rows, one per side
    DEADB = int(_remap(NPAD - 1))
    assert DEADA < SPLIT <= DEADB

    def side_arrays(t, side, C):
        """Slot-assign tile t's side edges into C chunks (pair-aware).

        side 0: edges with remapped col row < SPLIT; side 1: rest.
        Returns sidx[C*128] (int16 gather idx, dead rows for pads) and
        pair_dst[C*64] (local dst 0..127, -1 for fully dead pairs) and
        per-node half-dead-pair count [P]."""
        a0, a1 = bounds[t], bounds[t + 1]
        segs = seg_s[a0:a1].astype(np.int64)
        rows = _remap(col_s[a0:a1].astype(np.int64))
        m = (rows < SPLIT) if side == 0 else (rows >= SPLIT)
        segs = segs[m]
        rows = rows[m] - (0 if side == 0 else SPLIT)
        o2 = np.argsort(segs, kind="stable")
        segs = segs[o2]
        rows = rows[o2]
        counts = np.bincount(segs, minlength=P)
        padc = counts + (counts & 1)
        poffs = np.concatenate([[0], np.cumsum(padc)])
        offs_ = np.concatenate([[0], np.cumsum(counts)])
        rank = np.arange(len(segs)) - offs_[segs]
        ppos = poffs[segs] + rank
        S = C * P
        assert poffs[-1] <= S, (poffs[-1], S)
        q, h = ppos // 2, ppos % 2
        slots = P * (2 * (q // P) + h) + (q % P)
        dead = (DEADA if side == 0 else DEADB - SPLIT)
        sidx = np.full(S, dead, np.int64)
        sidx[slots] = rows
        pair_dst = np.full(S // 2, -1, np.int64)
        ev = (ppos % 2 == 0)
        pair_dst[ppos[ev] // 2] = segs[ev]
        return sidx, pair_dst, (counts & 1).astype(np.float32)

    def wrap16(idx):
        a = np.asarray(idx, np.int16)
        assert len(a) % 16 == 0
        return np.tile(np.ascontiguousarray(a.reshape(-1, 16).T), (8, 1))

    metas = []
    group_meta = None
    node_order = []
    for c in range(NCORES):
        parts = []
        offs = []
        off = 0
        ids_order = []
        for gi in range(NGROUPS):
            slots = group_slots[gi]
            G = len(slots)
            Cab = [(int(CsA[j]), int(CsB[j])) for j in slots]
            GCa = sum(a for a, b in Cab)
            GCb = sum(b for a, b in Cab)
            GC = GCa + GCb
            idxA = np.zeros(GCa * P, np.int64)
            idxB = np.zeros(GCb * P, np.int64)
            pairidx = np.zeros(GC * 64, np.int64)
            ohidx = np.full(GC * 64, P, np.int64)
            ninit = np.zeros((P, G), np.int32)
            padcnt = np.zeros((P, G), np.float32)
            abase = 0
            bbase = GCa
            for ti, j in enumerate(slots):
                t = core_tiles[c][j]
                cA, cB = Cab[ti]
                ids = tile_nodes[t]
                ids_order.append(ids)
                ninit[:, ti] = _remap(ids.astype(np.int64))
                for side, C, base in ((0, cA, abase), (1, cB, bbase)):
                    if C == 0:
                        continue
                    sidx, pair_dst, npad = side_arrays(t, side, C)
                    if side == 0:
                        idxA[abase * P:(abase + C) * P] = sidx
                    else:
                        idxB[(bbase - GCa) * P:(bbase - GCa + C) * P] = sidx
                    dead = pair_dst < 0
                    pairidx[base * 64:(base + C) * 64] = \
                        np.where(dead, 0, pair_dst * G + ti)
                    ohidx[base * 64:(base + C) * 64] = \
                        np.where(dead, P, pair_dst)
                    padcnt[:, ti] += npad
                abase += cA
                bbase += cB
            padcnt -= 1e-4            # -eps keeps denom > 0
            mt = np.concatenate(
                [wrap16(idxA), wrap16(idxB),
                 wrap16(pairidx), wrap16(ohidx),
                 ninit.view(np.int16).reshape(P, 2 * G),
                 padcnt.astype(bf16).view(np.int16)], 1)
            if mt.shape[1] % 2:
                mt = np.concatenate([mt, np.zeros((P, 1), np.int16)], 1)
            offs.append((off, G, Cab))
            off += mt.shape[1]
            parts.append(mt)
        metas.append(np.ascontiguousarray(np.concatenate(parts, 1)))
        node_order.append(np.concatenate(ids_order))
        if group_meta is None:
            group_meta = offs
        else:
            assert offs == group_meta
    assert all(m.shape == metas[0].shape for m in metas)
    return metas, group_meta, node_order


# --------------------------------------------------------------------------
# device program
# --------------------------------------------------------------------------

def _build(group_meta, with_bias, meta_w):
    f32, b16, i16, i32, fp8 = (mybir.dt.float32, mybir.dt.bfloat16,
                               mybir.dt.int16, mybir.dt.int32,
                               mybir.dt.float8e4)
    nc = bacc.Bacc()
    xt_in = nc.declare_dram_parameter("xt", [P, NPAD], f32, isOutput=False)
    w_in = nc.declare_dram_parameter("w", [F, F], f32, isOutput=False)
    meta_in = nc.declare_dram_parameter("meta", [P, meta_w], i16, isOutput=False)
    idtab_in = nc.declare_dram_parameter("idtab", [P + 8, 128], i16, isOutput=False)
    bias_in = nc.declare_dram_parameter("biasr", [P, F], f32, isOutput=False)
    out = nc.declare_dram_parameter("out", [P, TPC * F], f32, isOutput=True)

    h16 = nc.dram_tensor("h16", [NPAD, F], b16)   # partition-major rows

    AF = mybir.ActivationFunctionType
    OP = mybir.AluOpType

    with tile.TileContext(nc) as tc:
        with (
            tc.tile_pool(name="const", bufs=1) as constp,
            tc.tile_pool(name="meta", bufs=2 * FLIGHT + 1) as metap,
            tc.tile_pool(name="hntp", bufs=2 * FLIGHT + 1) as hntp,
            tc.tile_pool(name="ph0", bufs=3) as ph0p,
            tc.tile_pool(name="gpool", bufs=2 * FLIGHT - 1) as gp,
            tc.tile_pool(name="ohpool", bufs=2 * FLIGHT) as ohp,
            tc.tile_pool(name="hdpool", bufs=FLIGHT + 1) as hdp,
            tc.tile_pool(name="scp", bufs=FLIGHT + 1) as scp,
            tc.tile_pool(name="cmbp", bufs=FLIGHT) as cmbp,
            tc.tile_pool(name="small", bufs=FLIGHT) as sp,
            tc.tile_pool(name="psum0", bufs=2, space="PSUM") as psum0,
            tc.tile_pool(name="psumda", bufs=6, space="PSUM") as psumda,
            tc.tile_pool(name="dram", bufs=4 * FLIGHT, space="DRAM") as dramp,
        ):
            wt = constp.tile([F, F], f32)
            nc.sync.dma_start(out=wt[:], in_=w_in[:])
            eps_t = constp.tile([P, 1], f32)
            nc.vector.memset(eps_t[:], 1e-12)
            if with_bias:
                bias_t = constp.tile([P, F], f32)
                nc.sync.dma_start(out=bias_t[:], in_=bias_in[:])

            h16v = h16[:].rearrange("(p nb) f -> p nb f", p=P)

            # ---- phase 0: h16 = normalize_k(leaky_relu(x @ W [+ b]))
            NGRP = NPAD // (P * PGRP)           # 49 blocks of 1024
            for b in range(NGRP):
                xts = ph0p.tile([P, PGRP, F], f32, tag="xts")
                nc.sync.dma_start(
                    out=xts[:],
                    in_=xt_in[:, b * PGRP * P:(b + 1) * PGRP * P]
                        .rearrange("p (g n) -> p g n", g=PGRP))
                hp = psum0.tile([P, PGRP * F], f32, tag="hp")
                for j in range(PGRP):
                    nc.tensor.matmul(out=hp[:, j * F:(j + 1) * F],
                                     lhsT=xts[:, j, :], rhs=wt[:],
                                     start=True, stop=True)
                if with_bias:
                    hb = ph0p.tile([P, PGRP * F], f32, tag="hb")
                    nc.vector.tensor_tensor(
                        out=hb[:].rearrange("p (g n) -> p g n", g=PGRP),
                        in0=hp[:].rearrange("p (g n) -> p g n", g=PGRP),
                        in1=bias_t[:].unsqueeze(1).to_broadcast([P, PGRP, F]),
                        op=OP.add)
                    zsrc = hb
                else:
                    zsrc = hp
                # leaky_relu(z) = max(z, 0.01*z)
                r1 = ph0p.tile([P, PGRP * F], b16, tag="r1")
                nc.scalar.activation(out=r1[:], in_=zsrc[:], func=AF.Copy)
                z2 = ph0p.tile([P, PGRP * F], b16, tag="z2")
                nc.vector.tensor_scalar_mul(z2[:], r1[:], 0.01)
                h = r1
                nc.vector.tensor_tensor(out=h[:], in0=r1[:], in1=z2[:], op=OP.max)
                sq = z2
                nc.vector.tensor_tensor(out=sq[:], in0=h[:], in1=h[:],
                                        op=OP.mult)
                sqv = sq[:].rearrange("p (g d k) -> p g d k", g=PGRP, d=D)
                nc.vector.tensor_tensor(out=sqv[:, :, 0:8, :],
                                        in0=sqv[:, :, 0:8, :],
                                        in1=sqv[:, :, 8:16, :], op=OP.add)
                nc.vector.tensor_tensor(out=sqv[:, :, 0:4, :],
                                        in0=sqv[:, :, 0:4, :],
                                        in1=sqv[:, :, 4:8, :], op=OP.add)
                nc.vector.tensor_tensor(out=sqv[:, :, 0:2, :],
                                        in0=sqv[:, :, 0:2, :],
                                        in1=sqv[:, :, 2:4, :], op=OP.add)
                ss = ph0p.tile([P, PGRP, K], f32, tag="ss")
                nc.vector.tensor_tensor(out=ss[:].unsqueeze(2),
                                        in0=sqv[:, :, 0:1, :],
                                        in1=sqv[:, :, 1:2, :], op=OP.add)
                sroot = ph0p.tile([P, PGRP, K], f32, tag="sroot")
                nc.scalar.activation(out=sroot[:], in_=ss[:], func=AF.Sqrt,
                                     bias=eps_t[:])
                rsf = ph0p.tile([P, PGRP, K], f32, tag="rsf")
                nc.vector.reciprocal_approx_fast(out=rsf[:], in_=sroot[:])
                rs = ph0p.tile([P, PGRP, K], b16, tag="rs")
                nc.vector.tensor_copy(out=rs[:], in_=rsf[:])
                hn = sq
                nc.vector.tensor_tensor(
                    out=hn[:].rearrange("p (g d k) -> p g d k", g=PGRP, d=D),
                    in0=h[:].rearrange("p (g d k) -> p g d k", g=PGRP, d=D),
                    in1=rs[:].unsqueeze(2).to_broadcast([P, PGRP, D, K]),
                    op=OP.mult)
                nc.sync.dma_start(
                    out=h16v[:, b * PGRP:(b + 1) * PGRP, :],
                    in_=hn[:].rearrange("p (g f) -> p g f", g=PGRP))

            if DEBUG_STAGE == 1:
                zt = sp.tile([P, F // 2], i16, tag="zt")
                nc.vector.memset(zt[:], 0)
                for j in range(TPC * 2):
                    nc.sync.dma_start(
                        out=out[:, j * (F // 2):(j + 1) * (F // 2)],
                        in_=zt[:])

            # ---- iterations, groups processed in pairs for overlap
            tbase_of = []
            tb = 0
            for off, G, Cls in group_meta:
                tbase_of.append(tb)
                tb += G

            def group_init(gmeta, tbase):
                off, G, Cab = gmeta
                GCa = sum(a for a, b in Cab)
                GCb = sum(b for a, b in Cab)
                GC = GCa + GCb
                W_t = 8 * GCa + 8 * GCb + 4 * GC + 4 * GC + 2 * G + G
                W_t += W_t % 2
                mt = metap.tile([P, W_t], i16, tag="mt")
                nc.sync.dma_start(out=mt[:], in_=meta_in[:, off:off + W_t])
                o = 0
                idxA_t = mt[:, o:o + 8 * GCa]; o += 8 * GCa
                idxB_t = mt[:, o:o + 8 * GCb]; o += 8 * GCb
                pairidx_t = mt[:, o:o + 4 * GC]; o += 4 * GC
                ohidx_t = mt[:, o:o + 4 * GC]; o += 4 * GC
                ninit_t = mt[:, o:o + 2 * G].bitcast(i32); o += 2 * G
                padc_t = mt[:, o:o + G].bitcast(b16); o += G

                hnt = hntp.tile([P, G, F], b16, tag="hnt")
                for ti in range(G):
                    nc.gpsimd.indirect_dma_start(
                        out=hnt[:, ti, :], out_offset=None, in_=h16[:],
                        in_offset=bass.IndirectOffsetOnAxis(
                            ap=ninit_t[:, ti:ti + 1], axis=0))
                hntd = sp.tile([P, G, 2, F], b16, tag="hntd")
                nc.vector.tensor_copy(
                    out=hntd[:],
                    in_=hnt[:].unsqueeze(2).to_broadcast([P, G, 2, F]))
                bounce = dramp.tile([P, G * 2 * F], b16, tag="bounce")
                nc.sync.dma_start(out=bounce[:], in_=hntd[:])

                g = gp.tile([P, GC, F], b16, tag="g")
                if GCa:
                    nc.gpsimd.dma_gather(
                        out_ap=g[:, 0:GCa, :], in_ap=h16[0:32768, :],
                        idxs_ap=idxA_t, num_idxs=GCa * P,
                        num_idxs_reg=GCa * P, elem_size=F,
                        single_packet=False)
                if GCb:
                    nc.gpsimd.dma_gather(
                        out_ap=g[:, GCa:GC, :], in_ap=h16[32768:NPAD, :],
                        idxs_ap=idxB_t, num_idxs=GCb * P,
                        num_idxs_reg=GCb * P, elem_size=F,
                        single_packet=False)
                oh2 = ohp.tile([P, GC // 2, 256], fp8, tag="oh")
                nc.gpsimd.dma_gather(
                    out_ap=oh2[:], in_ap=idtab_in[:].bitcast(fp8),
                    idxs_ap=ohidx_t, num_idxs=GC * 64, num_idxs_reg=GC * 64,
                    elem_size=256, single_packet=False)
                return dict(G=G, GC=GC, GCa=GCa, Cab=Cab, pairidx=pairidx_t,
                            padc=padc_t, bounce=bounce, hnt=hnt, g=g, oh=oh2,
                            tbase=tbase)

            def pair_iter(sts, it):
                """One iteration for a pair of groups, engine-phase
                interleaved so ACT alternates tables only twice."""
                last_it = it == ITERS - 1
                # Pool: h_dst expansion gathers
                for st in sts:
                    GC = st["GC"]
                    hdexp2 = hdp.tile([P, GC // 2, 2 * F], b16, tag="hdexp")
                    nc.gpsimd.dma_gather(
                        out_ap=hdexp2[:],
                        in_ap=st["bounce"][:].rearrange("p (r f) -> (p r) f",
                                                        f=2 * F),
                        idxs_ap=st["pairidx"], num_idxs=GC * 64,
                        num_idxs_reg=GC * 64, elem_size=2 * F,
                        single_packet=False)
                    st["hdexp"] = hdexp2[:].rearrange(
                        "p c2 (h f) -> p (c2 h) f", h=2)
                # DVE: scores (prod in-place into hdexp, then d-tree)
                for st in sts:
                    GC, g, hdexp = st["GC"], st["g"], st["hdexp"]
                    nc.vector.tensor_tensor(out=hdexp, in0=g[:, 0:GC, :],
                                            in1=hdexp, op=OP.mult)
                    pv = hdexp.rearrange("p c (d k) -> p c d k", d=D)
                    nc.vector.tensor_tensor(out=pv[:, :, 0:8, :],
                                            in0=pv[:, :, 0:8, :],
                                            in1=pv[:, :, 8:16, :], op=OP.add)
                    nc.vector.tensor_tensor(out=pv[:, :, 0:4, :],
                                            in0=pv[:, :, 0:4, :],
                                            in1=pv[:, :, 4:8, :], op=OP.add)
                    nc.vector.tensor_tensor(out=pv[:, :, 0:2, :],
                                            in0=pv[:, :, 0:2, :],
                                            in1=pv[:, :, 2:4, :], op=OP.add)
                    sc = scp.tile([P, GC, K], f32, tag="sc")
                    nc.vector.tensor_tensor(out=sc[:].unsqueeze(2),
                                            in0=pv[:, :, 0:1, :],
                                            in1=pv[:, :, 1:2, :], op=OP.add)
                    st["sc"] = sc
                # ACT: exp (table: exp_and_others)
                for st in sts:
                    GC = st["GC"]
                    cmb = cmbp.tile([P, GC, K + F], b16, tag="cmb")
                    nc.scalar.activation(out=cmb[:, :, 0:K], in_=st["sc"][:],
                                         func=AF.Exp)
                    st["cmb"] = cmb
                # DVE: weighted sources
                for st in sts:
                    GC, g, cmb = st["GC"], st["g"], st["cmb"]
                    nc.vector.tensor_tensor(
                        out=cmb[:, :, K:K + F].rearrange(
                            "p c (d k) -> p c d k", d=D),
                        in0=g[:, 0:GC, :].rearrange("p c (d k) -> p c d k",
                                                    d=D),
                        in1=cmb[:, :, 0:K].unsqueeze(2)
                            .to_broadcast([P, GC, D, K]),
                        op=OP.mult)
                # PE: denom + aggregation matmuls, ACT copies (table-neutral)
                for st in sts:
                    G, Cab, cmb = st["G"], st["Cab"], st["cmb"]
                    GCa = st["GCa"]
                    oh = st["oh"][:].rearrange("p c2 (h n) -> p (c2 h) n", h=2)
                    dacp = sp.tile([P, G, K + F], f32, tag="dacp")
                    abase = 0
                    bbase = GCa
                    for ti in range(G):
                        cA, cB = Cab[ti]
                        chunks = ([abase + i for i in range(cA)] +
                                  [bbase + i for i in range(cB)])
                        da = psumda.tile([P, K + F], f32, tag="da")
                        for j, cc in enumerate(chunks):
                            nc.tensor.matmul(out=da[:],
                                             lhsT=oh[:, cc, :],
                                             rhs=cmb[:, cc, :],
                                             start=(j == 0),
                                             stop=(j == len(chunks) - 1))
                        nc.scalar.activation(out=dacp[:, ti, :],
                                             in_=da[:], func=AF.Copy)
                        abase += cA
                        bbase += cB
                    st["dacp"] = dacp
                # DVE: denom fixup + 1/denom + attr
                for st in sts:
                    G, dacp = st["G"], st["dacp"]
                    deps = sp.tile([P, G, K], f32, tag="deps")
                    nc.vector.tensor_tensor(
                        out=deps[:], in0=dacp[:, :, 0:K],
                        in1=st["padc"].unsqueeze(-1).to_broadcast([P, G, K]),
                        op=OP.subtract)
                    rdenf = sp.tile([P, G, K], f32, tag="rdenf")
                    nc.vector.reciprocal_approx_fast(out=rdenf[:], in_=deps[:])
                    attr = sp.tile([P, G, F], f32, tag="attr")
                    nc.vector.tensor_tensor(
                        out=attr[:].rearrange("p g (d k) -> p g d k", d=D),
                        in0=dacp[:, :, K:K + F].rearrange(
                            "p g (d k) -> p g d k", d=D),
                        in1=rdenf[:].unsqueeze(2).to_broadcast([P, G, D, K]),
                        op=OP.mult)
                    nc.vector.tensor_tensor(out=attr[:], in0=attr[:],
                                            in1=st["hnt"][:], op=OP.add)
                    st["attr"] = attr
                # ACT: Square (table-neutral)
                for st in sts:
                    G = st["G"]
                    asq = sp.tile([P, G, F], f32, tag="asq")
                    nc.scalar.activation(out=asq[:], in_=st["attr"][:],
                                         func=AF.Square)
                    st["asq"] = asq
                # DVE: |attr|^2 tree
                for st in sts:
                    G = st["G"]
                    aqv = st["asq"][:].rearrange("p g (d k) -> p g d k", d=D)
                    nc.vector.tensor_tensor(out=aqv[:, :, 0:8, :],
                                            in0=aqv[:, :, 0:8, :],
                                            in1=aqv[:, :, 8:16, :], op=OP.add)
                    nc.vector.tensor_tensor(out=aqv[:, :, 0:4, :],
                                            in0=aqv[:, :, 0:4, :],
                                            in1=aqv[:, :, 4:8, :], op=OP.add)
                    nc.vector.tensor_tensor(out=aqv[:, :, 0:2, :],
                                            in0=aqv[:, :, 0:2, :],
                                            in1=aqv[:, :, 2:4, :], op=OP.add)
                    ss2 = sp.tile([P, G, K], f32, tag="ss2")
                    nc.vector.tensor_tensor(out=ss2[:].unsqueeze(2),
                                            in0=aqv[:, :, 0:1, :],
                                            in1=aqv[:, :, 1:2, :], op=OP.add)
                    st["ss2"] = ss2
                # ACT: Sqrt (table: sqrt_and_others)
                for st in sts:
                    G = st["G"]
                    sroot2 = sp.tile([P, G, K], f32, tag="sroot2")
                    nc.scalar.activation(out=sroot2[:], in_=st["ss2"][:],
                                         func=AF.Sqrt, bias=eps_t[:])
                    st["sroot2"] = sroot2
                # DVE: rsqrt + renormalized output
                for st in sts:
                    G, attr = st["G"], st["attr"]
                    rs2 = sp.tile([P, G, K], f32, tag="rs2")
                    nc.vector.reciprocal_approx_fast(out=rs2[:],
                                                     in_=st["sroot2"][:])
                    if not last_it:
                        hnew = sp.tile([P, G, F], b16, tag="hnew")
                        nc.vector.tensor_tensor(
                            out=hnew[:].rearrange("p g (d k) -> p g d k", d=D),
                            in0=attr[:].rearrange("p g (d k) -> p g d k", d=D),
                            in1=rs2[:].unsqueeze(2).to_broadcast([P, G, D, K]),
                            op=OP.mult)
                        hnewd = sp.tile([P, G, 2, F], b16, tag="hnewd")
                        nc.vector.tensor_copy(
                            out=hnewd[:],
                            in_=hnew[:].unsqueeze(2).to_broadcast([P, G, 2, F]))
                        bounce = dramp.tile([P, G * 2 * F], b16, tag="bounce")
                        nc.sync.dma_start(out=bounce[:], in_=hnewd[:])
                        st["bounce"] = bounce
                    else:
                        outv = sp.tile([P, G, F], f32, tag="outv")
                        nc.vector.tensor_tensor(
                            out=outv[:].rearrange("p g (d k) -> p g d k", d=D),
                            in0=attr[:].rearrange("p g (d k) -> p g d k", d=D),
                            in1=rs2[:].unsqueeze(2).to_broadcast([P, G, D, K]),
                            op=OP.mult)
                        tb = st["tbase"]
                        nc.sync.dma_start(
                            out=out[:, tb * F:(tb + G) * F],
                            in_=outv[:].rearrange("p g f -> p (g f)"))

            if DEBUG_STAGE != 1:
                windows = [list(range(p0, min(p0 + FLIGHT, NGROUPS)))
                           for p0 in range(0, NGROUPS, FLIGHT)]
                if len(windows) > 1 and len(windows[-1]) == 1:
                    windows[-2].extend(windows.pop())
                sts_next = [group_init(group_meta[gi], tbase_of[gi])
                            for gi in windows[0][:FLIGHT]]
                carry = windows[0][FLIGHT:]
                for wi, win in enumerate(windows):
                    sts = sts_next + [group_init(group_meta[gi], tbase_of[gi])
                                      for gi in carry]
                    sts_next = None
                    for it in range(ITERS):
                        if it == ITERS - 1 and wi + 1 < len(windows):
                            nxt = windows[wi + 1]
                            sts_next = [group_init(group_meta[gi],
                                                   tbase_of[gi])
                                        for gi in nxt[:FLIGHT]]
                            carry = nxt[FLIGHT:]
                        pair_iter(sts, it)
    if not nc.is_finalized():
        nc.finalize()
    return nc


# --------------------------------------------------------------------------
# entry point
# --------------------------------------------------------------------------

def kernel(x, edge_index, weight, bias):
    x = np.asarray(x, dtype=np.float32)
    weight = np.asarray(weight, dtype=np.float32)
    bias = np.asarray(bias, dtype=np.float32)
    assert x.shape == (N_NODES, F) and edge_index.shape == (2, N_EDGES)

    metas, group_meta, node_order = _preprocess(edge_index)
    with_bias = bool(np.any(bias != 0))
    nc = _build(group_meta, with_bias, metas[0].shape[1])

    # device feature order: f' = d*K + k  <->  f = k*D + d
    perm = np.array([k * D + d for d in range(D) for k in range(K)])
    xpad = np.zeros((NPAD, F), np.float32)
    xpad[:N_NODES] = x
    xt = np.ascontiguousarray(xpad.T)                       # [128, NPAD] f32
    wp = np.ascontiguousarray(weight[:, perm])
    biasr = np.tile(bias[perm].astype(np.float32)[None, :], (P, 1))

    # fp8 doubled identity table: row r<128 = onehot(r) doubled; row 128+ = 0
    idt = np.zeros((P + 8, 256), f8e4)
    one = f8e4(1.0)
    for r in range(P):
        idt[r, r] = one
        idt[r, 128 + r] = one
    idtab = np.ascontiguousarray(idt).view(np.int16)

    in_maps = [
        dict(xt=xt, w=wp, meta=metas[c], idtab=idtab, biasr=biasr)
        for c in range(NCORES)
    ]
    res = run_bass_kernel_spmd(nc, in_maps, core_ids=list(range(NCORES)),
                               trace=TRACE)
    LAST_RESULTS["exec_time_ns"] = res.exec_time_ns
    LAST_RESULTS["trace"] = res.instructions_and_trace
    LAST_RESULTS["nc"] = nc
    LAST_RESULTS["in_maps"] = in_maps

    full = np.zeros((NPAD, F), np.float32)
    for c in range(NCORES):
        o = res.results[c]["out"].astype(np.float32)
        o = o.reshape(P, TPC, F).transpose(1, 0, 2).reshape(NPC, F)
        full[node_order[c][:, None], perm[None, :]] = o
    return full[:N_NODES]

